# revision 24
# baseline (speedup 1.0000x reference)
"""Block-diagonal grouped conv2d (64 heads, 4->4 ch each, 3x3, pad 1) on 8 trn2 cores.

Strategy:
- Data-parallel over batch: 4 images per core, no collectives.
- Channels -> SBUF partitions. Half hf = heads [32hf, 32hf+32); within a
  half, quartet group g in {0,1} = heads [32hf+16g, +16). Partition
  layout p = 64g + ic*16 + hh (hh = head within group), so every DMA is
  a contiguous 16-partition slice of 16 contiguous DRAM channels
  (channel c = ic*64 + head).
- Conv as 9 shifted matmuls accumulated in PSUM. Four 64x64 matmuls run
  concurrently in the PE array via tile_position quadrants:
    hf0: g0 -> (0,0),  g1 -> (64,64);  hf1: g0 -> (0,64), g1 -> (64,0)
  Each 64x64 stationary holds 16 heads' 4x4 blocks (permuted diag),
  built on the host.
- fp16 compute (1 col/cycle PE, fast weight load); x is DMA'd as f32
  and cast to fp16 on the vector engine. PSUM accumulates in f32.
- Rows padded to 130 cols (zero borders, pad-only memsets) so x-shifts
  stay in-row; strips of 32 rows with 1-row halo; 416-wide chunks.
- Input DMAs on the SP HWDGE ring, output DMAs on the ACT ring.
"""

import numpy as np

import concourse.bass as bass
import concourse.bacc as bacc
import concourse.mybir as mybir
from concourse.tile import TileContext
from concourse.bass_utils import run_bass_kernel_spmd

# problem shapes (hardcoded per harness contract)
B, CIN, H, W = 32, 256, 128, 128
M, CPO, CPI = 64, 4, 4
NCORES = 8
BC = B // NCORES          # images per core
R = 32                    # output rows per strip
HALO = R + 2              # input rows per strip
WP = W + 2                # padded row width
NSTRIP = H // R
CHUNK = 416               # matmul free dim: 10*416 == R*WP
NCHUNK = (R * WP) // CHUNK
FIN = HALO * WP + 2       # in-tile flat size (+1 guard elem each end)
FOUT = R * WP

F32 = mybir.dt.float32
FP16 = mybir.dt.float16

OFFS = [(dy, dx) for dy in (-1, 0, 1) for dx in (-1, 0, 1)]

_cache = {}


def _build_nc(repeat: int):
    nc = bacc.Bacc("TRN2", target_bir_lowering=False, debug=False,
                   num_devices=NCORES)
    x_d = nc.dram_tensor("x", (BC, CIN, H, W), F32, kind="ExternalInput").ap()
    w_d = nc.dram_tensor("wstack", (18, 128, 64), FP16,
                         kind="ExternalInput").ap()
    b_d = nc.dram_tensor("bias2", (128, 2), F32, kind="ExternalInput").ap()
    o_d = nc.dram_tensor("out", (BC, CIN, H, W), F32, kind="ExternalOutput").ap()

    with TileContext(nc) as tc:
        with tc.tile_pool(name="wpool", bufs=1) as wpool, \
             tc.tile_pool(name="xin", bufs=2) as xinp, \
             tc.tile_pool(name="xh", bufs=2) as xhp, \
             tc.tile_pool(name="xout", bufs=2) as xoutp, \
             tc.tile_pool(name="psum", bufs=4, space="PSUM") as psp:

            wsb = wpool.tile([128, 18 * 64], FP16)
            for t in range(18):
                nc.sync.dma_start(
                    out=wsb[:, t * 64:(t + 1) * 64], in_=w_d[t])
            bsb = wpool.tile([128, 2], F32)
            nc.sync.dma_start(out=bsb[:], in_=b_d)

            for rep in range(repeat):
                for b in range(BC):
                    for s in range(NSTRIP):
                        y0 = s * R
                        # valid input rows [ry0, ry1) of image; tile row 0 is y0-1
                        ry0 = max(y0 - 1, 0)
                        ry1 = min(y0 + R + 1, H)
                        r_lo = ry0 - (y0 - 1)
                        r_hi = ry1 - (y0 - 1)
                        xbs = []
                        for hf in range(2):
                            xt = xinp.tile([128, FIN], F32, tag=f"xin{hf}")
                            x3 = xt[:, 1:1 + HALO * WP].rearrange(
                                "p (r c) -> p r c", c=WP)
                            # zero pads: guard elems, pad columns, halo rows
                            nc.gpsimd.memset(xt[:, 0:1], 0.0)
                            nc.gpsimd.memset(xt[:, FIN - 1:FIN], 0.0)
                            nc.gpsimd.memset(x3[:, :, 0:1], 0.0)
                            nc.gpsimd.memset(x3[:, :, WP - 1:WP], 0.0)
                            if r_lo > 0:
                                nc.gpsimd.memset(x3[:, 0:r_lo, :], 0.0)
                            if r_hi < HALO:
                                nc.gpsimd.memset(x3[:, r_hi:HALO, :], 0.0)
                            for g in range(2):
                                for i in range(CPI):
                                    nc.sync.dma_start(
                                        out=x3[64 * g + 16 * i:
                                               64 * g + 16 * i + 16,
                                               r_lo:r_hi, 1:1 + W],
                                        in_=x_d[b, i * 64 + 32 * hf + 16 * g:
                                                i * 64 + 32 * hf + 16 * g + 16,
                                                ry0:ry1, :])
                            xb = xhp.tile([128, FIN], FP16, tag=f"xh{hf}")
                            nc.vector.tensor_copy(xb[:], xt[:])
                            xbs.append(xb)

                        ots = [xoutp.tile([128, FOUT], F32, tag=f"xout{hf}",
                                          name=f"ot{hf}")
                               for hf in range(2)]
                        for c in range(NCHUNK):
                            c0 = c * CHUNK
                            pts = [psp.tile([128, CHUNK], F32, tag=f"pt{hf}",
                                           name=f"pt{hf}")
                                   for hf in range(2)]
                            for t, (dy, dx) in enumerate(OFFS):
                                src = 1 + c0 + WP + dy * WP + dx
                                for hf in range(2):
                                    for g in range(2):
                                        q = g if hf == 0 else 1 - g
                                        nc.tensor.matmul(
                                            pts[hf][64 * q:64 * q + 64, :],
                                            wsb[64 * g:64 * g + 64,
                                                (hf * 9 + t) * 64:
                                                (hf * 9 + t + 1) * 64],
                                            xbs[hf][64 * g:64 * g + 64,
                                                    src:src + CHUNK],
                                            start=(t == 0), stop=(t == 8),
                                            skip_group_check=True)
                            for hf in range(2):
                                nc.scalar.activation(
                                    ots[hf][:, c0:c0 + CHUNK], pts[hf][:],
                                    mybir.ActivationFunctionType.Identity,
                                    bias=bsb[:, hf:hf + 1])
                        for hf in range(2):
                            o3 = ots[hf].rearrange("p (r c) -> p r c", c=WP)
                            for g in range(2):
                                q = g if hf == 0 else 1 - g
                                for o in range(CPO):
                                    nc.scalar.dma_start(
                                        out=o_d[b, o * 64 + 32 * hf + 16 * g:
                                                o * 64 + 32 * hf + 16 * g + 16,
                                                y0:y0 + R, :],
                                        in_=o3[64 * q + 16 * o:
                                               64 * q + 16 * o + 16,
                                               :, 1:1 + W])
    nc.compile()
    return nc


def _prep_weights(weights: np.ndarray) -> np.ndarray:
    # wstack[hf*9+t][64g + ic*16 + hh, oc*16 + hh] =
    #     weights[32hf + 16g + hh, oc, ic, dy, dx]
    ws = np.zeros((2, 9, 2, CPI, 16, CPO, 16), dtype=np.float32)
    wr = np.asarray(weights, dtype=np.float32).reshape(2, 2, 16, CPO, CPI, 3, 3)
    ar = np.arange(16)
    for t, (dy, dx) in enumerate(OFFS):
        for ic in range(CPI):
            for oc in range(CPO):
                # ws[hf, t, g, ic, hh, oc, hh] = wr[hf, g, hh, oc, ic, ky, kx]
                ws[:, t, :, ic, ar, oc, ar] = \
                    wr[:, :, :, oc, ic, dy + 1, dx + 1].transpose(2, 0, 1)
    return ws.reshape(2, 9, 128, 64).reshape(18, 128, 64).astype(np.float16)


def _prep_bias(bias: np.ndarray) -> np.ndarray:
    # psum partition p = 64q + oc*16 + hh holds head 32hf + 16g(q,hf) + hh
    # with g = q for hf=0, g = 1-q for hf=1
    b2 = np.zeros((128, 2), dtype=np.float32)
    br = np.asarray(bias, dtype=np.float32).reshape(2, 2, 16, CPO)  # hf,g,hh,oc
    for hf in range(2):
        for q in range(2):
            g = q if hf == 0 else 1 - q
            for oc in range(CPO):
                b2[64 * q + oc * 16:64 * q + oc * 16 + 16, hf] = br[hf, g, :, oc]
    return b2


def _get_nc(repeat: int):
    if repeat not in _cache:
        _cache[repeat] = _build_nc(repeat)
    return _cache[repeat]


def _run(x, weights, bias, repeat=1):
    nc = _get_nc(repeat)
    ws = _prep_weights(np.asarray(weights, dtype=np.float32))
    b2 = _prep_bias(np.asarray(bias, dtype=np.float32))
    x = np.asarray(x, dtype=np.float32)
    in_maps = [
        {"x": x[c * BC:(c + 1) * BC], "wstack": ws, "bias2": b2}
        for c in range(NCORES)
    ]
    res = run_bass_kernel_spmd(nc, in_maps, core_ids=list(range(NCORES)))
    return np.concatenate([res.results[c]["out"] for c in range(NCORES)],
                          axis=0)


def kernel(x, weights, bias):
    return _run(x, weights, bias, repeat=1)
